# revision 4
# baseline (speedup 1.0000x reference)
"""Trainium2 Bass kernel for nn_DivTree (moe_routing) — v7.

v6 -> v7 refinements:
- Warmup matmuls alternate two PSUM banks (100% PE duty) so the HAM
  clock-gate reliably lifts to 2.4 GHz on the first activity window;
  the single-bank WAW chain only reached ~50% duty and sometimes left
  the head running at 1.2 GHz.
- W1 prefetch pacing dep moved one agent later (the group-1 SWDGE
  burst was still stealing SDMA quantum from the x0 queues at ~20us).
- Final expert group's L3 runs in two batch halves so the last
  bias-add + output store overlap the last matmuls (shorter tail).
"""

import numpy as np

P = 128
N_CORES = 8

_cache: dict = {}


def _build(A, D, H, NA, Bl, groups):
    import concourse.mybir as mybir
    import concourse.tile as tile
    from concourse import bacc
    from contextlib import ExitStack

    f32 = mybir.dt.float32
    bf16 = mybir.dt.bfloat16
    Relu = mybir.ActivationFunctionType.Relu
    E = len(groups)
    G = len(groups[0])
    assert all(len(g) == G for g in groups) and G * NA == P
    KD, KH, MH = D // P, H // P, H // P
    NB = Bl
    assert NB <= 512 and H % P == 0 and D % P == 0 and NA <= P

    nc = bacc.Bacc()
    x0t = nc.declare_dram_parameter("x0t", [A, D, Bl], bf16, isOutput=False)
    ws = nc.declare_dram_parameter("ws", [D, H], bf16, isOutput=False)
    bs = nc.declare_dram_parameter("bs", [H], f32, isOutput=False)
    w1g = nc.declare_dram_parameter("w1g", [E, H, H], bf16, isOutput=False)
    b1g = nc.declare_dram_parameter("b1g", [E, H], f32, isOutput=False)
    w2g = nc.declare_dram_parameter("w2g", [E, H, NA], bf16, isOutput=False)
    b2q = nc.declare_dram_parameter("b2q", [E, P, 1], f32, isOutput=False)
    ytq = nc.declare_dram_parameter("ytq", [E, P, Bl], f32, isOutput=True)

    with tile.TileContext(nc) as tc, ExitStack() as ctx:
        const = ctx.enter_context(tc.tile_pool(name="const", bufs=1))
        wpool = ctx.enter_context(tc.tile_pool(name="wexp", bufs=2))
        xpool = ctx.enter_context(tc.tile_pool(name="x0", bufs=4))
        x1pool = ctx.enter_context(tc.tile_pool(name="x1", bufs=3))
        hpool = ctx.enter_context(tc.tile_pool(name="h", bufs=6))
        opool = ctx.enter_context(tc.tile_pool(name="out", bufs=2))
        psum = ctx.enter_context(tc.tile_pool(name="ps", bufs=4, space="PSUM"))
        psum2 = ctx.enter_context(tc.tile_pool(name="ps2", bufs=3, space="PSUM"))
        psum3 = ctx.enter_context(tc.tile_pool(name="ps3", bufs=1, space="PSUM"))

        # PE warm-up through the DMA cold-start. Two alternating PSUM
        # banks keep the dummy matmuls back-to-back (100% duty) so the
        # HAM activity window reliably sees a busy PE and lifts the clock
        # to 2.4 GHz before real work; a single bank's WAW serialization
        # gave only ~50% duty and sometimes failed the busy threshold.
        dummy = const.tile([P, 128], bf16)
        nc.vector.memset(dummy[:], 0.0)
        dwa = psum.tile([64, 128], f32, tag="ps", name="warm_a")
        dwb = psum.tile([64, 128], f32, tag="ps", name="warm_b")
        for i in range(40):
            dps = dwa if i % 2 == 0 else dwb
            nc.tensor.matmul(dps[:], dummy[:, :64], dummy[:, :128],
                             start=True, stop=True)

        # head-critical loads: pair k = (x0 piece k, wsm tile k) split
        # across the two HWDGE queues so both members land together
        ws_r = ws.rearrange("(ks p) h -> p ks h", p=P)
        a0 = groups[0][0]
        x0_first = xpool.tile([P, KD, NB], bf16, tag="x0")
        x0_first_r = x0t[a0].rearrange("(ks p) b -> p ks b", p=P)
        wsm = [const.tile([P, KD, P], bf16, tag=f"wsm{ms}", name=f"wsm{ms}")
               for ms in range(MH)]
        for k in range(KD):
            e_x, e_w = ((nc.sync, nc.scalar) if k % 2 == 0
                        else (nc.scalar, nc.sync))
            e_x.dma_start(x0_first[:, k, :], x0_first_r[:, k, :])
            e_w.dma_start(wsm[k][:], ws_r[:, :, k * P:(k + 1) * P])
        bs_t = const.tile([P, MH], f32)
        nc.scalar.dma_start(bs_t[:], bs.rearrange("(ms p) -> p ms", p=P))

        def emit_l1(a, x0_t, diagonal=False):
            x1_t = x1pool.tile([P, MH, NB], bf16, tag="x1", name=f"x1_{a}")
            ps1 = [psum.tile([P, NB], f32, tag="ps", name=f"ps1_{a}_{ms}")
                   for ms in range(MH)]
            if diagonal:
                order = [(ks, ms) for k in range(max(KD, MH))
                         for ks in range(KD) for ms in range(MH)
                         if max(ks, ms) == k]
            else:
                order = [(ks, ms) for ms in range(MH) for ks in range(KD)]
            for ks, ms in order:
                nc.tensor.matmul(
                    ps1[ms][:], wsm[ms][:, ks, :], x0_t[:, ks, :],
                    start=(ks == 0), stop=(ks == KD - 1),
                )
            for ms in range(MH):
                if ms % 2:
                    nc.vector.tensor_scalar(
                        x1_t[:, ms, :], ps1[ms][:], bs_t[:, ms:ms + 1], 0.0,
                        mybir.AluOpType.add, mybir.AluOpType.max)
                else:
                    nc.scalar.activation(x1_t[:, ms, :], ps1[ms][:], Relu,
                                         bias=bs_t[:, ms:ms + 1])
            return x1_t

        def emit_l2(a, x1_t, wt):
            w1_t, b1_t = wt[0], wt[1]
            h_t = hpool.tile([P, MH, NB], bf16, tag="h", name=f"h_{a}")
            for ms in range(MH):
                ps2 = psum2.tile([P, NB], f32, tag="ps2", name=f"ps2_{a}_{ms}")
                for ks in range(KH):
                    nc.tensor.matmul(
                        ps2[:],
                        w1_t[:, ks, ms * P:(ms + 1) * P],
                        x1_t[:, ks, :],
                        start=(ks == 0), stop=(ks == KH - 1),
                    )
                if ms % 2:
                    nc.vector.tensor_scalar(
                        h_t[:, ms, :], ps2[:], b1_t[:, ms:ms + 1], 0.0,
                        mybir.AluOpType.add, mybir.AluOpType.max)
                else:
                    nc.scalar.activation(h_t[:, ms, :], ps2[:], Relu,
                                         bias=b1_t[:, ms:ms + 1])
            return h_t

        def emit_l3_quad(s, h_ts, wt, split=False):
            w2_t, b2_t = wt[2], wt[3]
            ps3 = psum3.tile([P, NB], f32, tag="ps3", name=f"ps3_{s}")
            o_t = opool.tile([P, NB], f32, tag="o", name=f"o_{s}")
            # split=True pipelines the final quad in two batch halves so
            # the first half's bias-add + store overlap the second half's
            # matmuls (shrinks the post-matmul tail of the kernel)
            nh = 2 if split else 1
            hw = NB // nh
            for hf in range(nh):
                cl, cr = hf * hw, (hf + 1) * hw
                for ks in range(KH):
                    for j in range(G):
                        nc.tensor.matmul(
                            ps3[j * NA:(j + 1) * NA, cl:cr],
                            w2_t[:, ks, :],
                            h_ts[j][:, ks, cl:cr],
                            start=(ks == 0), stop=(ks == KH - 1),
                            tile_position=(0, j * NA),
                        )
                nc.vector.tensor_add(
                    o_t[:, cl:cr], ps3[:, cl:cr],
                    b2_t[:, 0:1].to_broadcast((P, hw)))
                nc.sync.dma_start(ytq[s][:, cl:cr], o_t[:, cl:cr])

        pending = None
        group_h = {}
        x0_tiles = [None] * A
        agents_flat = [a for g in groups for a in g]
        for s, agents in enumerate(groups):
            # SWDGE weight prefetch, paced: wait until an earlier agent's
            # x0 has landed so the deep W1 ring can't starve the HWDGE
            # queues during the head
            if s == 0:
                dep_t = x0_first
            else:
                dep_a = agents_flat[max(0, s * G - 1)]
                dep_t = x0_tiles[dep_a]
            w1_t = wpool.tile([P, KH, H], bf16, tag="w1", name=f"w1_{s}")
            w1_r = w1g[s].rearrange("(ks p) h -> p ks h", p=P)
            # WAW pacing: write one element of the destination tile from a
            # copy that reads the dep x0 tile's last-arriving slice. The
            # weight DMA (whole-tile write) must then order after the copy,
            # so the deep SWDGE ring can't start until the dep has landed.
            nc.gpsimd.tensor_copy(w1_t[0:1, 0, 0:1], dep_t[0:1, KD - 1, 0:1])
            nc.gpsimd.dma_start(w1_t[:], w1_r)
            w2_t = wpool.tile([P, KH, NA], bf16, tag="w2", name=f"w2_{s}")
            nc.gpsimd.tensor_copy(w2_t[0:1, 0, 0:1], dep_t[0:1, KD - 1, 0:1])
            nc.gpsimd.dma_start(
                w2_t[:], w2g[s].rearrange("(ks p) n -> p ks n", p=P))
            b1_t = wpool.tile([P, MH], f32, tag="b1", name=f"b1_{s}")
            b2_t = wpool.tile([P, 1], f32, tag="b2", name=f"b2_{s}")
            wt = (w1_t, b1_t, w2_t, b2_t)

            for ai, a in enumerate(agents):
                if a == a0:
                    x0_t = x0_first
                else:
                    x0_t = xpool.tile([P, KD, NB], bf16, tag="x0",
                                      name=f"x0_{a}")
                    x0_r = x0t[a].rearrange("(ks p) b -> p ks b", p=P)
                    half = KD // 2
                    nc.sync.dma_start(x0_t[:, :half, :], x0_r[:, :half, :])
                    nc.scalar.dma_start(x0_t[:, half:, :], x0_r[:, half:, :])
                x0_tiles[a] = x0_t
                if ai == 0:
                    # bias loads ride behind the first x0 trigger of the
                    # group so they never delay head-critical transfers
                    nc.scalar.dma_start(
                        b1_t[:], b1g[s].rearrange("(ms p) -> p ms", p=P))
                    nc.scalar.dma_start(b2_t[:], b2q[s])
                x1_t = emit_l1(a, x0_t, diagonal=(a == a0))
                if pending is not None:
                    pa, px1, pwt, pg, pj = pending
                    h_t = emit_l2(pa, px1, pwt)
                    group_h.setdefault(pg, []).append(h_t)
                    if pj == len(groups[pg]) - 1:
                        emit_l3_quad(pg, group_h.pop(pg), pwt)
                pending = (a, x1_t, wt, s, agents.index(a))
        pa, px1, pwt, pg, pj = pending
        h_t = emit_l2(pa, px1, pwt)
        group_h.setdefault(pg, []).append(h_t)
        emit_l3_quad(pg, group_h.pop(pg), pwt, split=True)

    nc.compile()
    return nc


def kernel(x0, W_shared, b_shared, W1, b1, W2, b2, route,
           _trace=False, _tmpdir=None):
    import ml_dtypes
    from concourse.bass_utils import run_bass_kernel_spmd

    bf = ml_dtypes.bfloat16
    x0 = np.asarray(x0, dtype=np.float32)
    route = np.asarray(route)

    B, A, D = x0.shape
    H = np.asarray(W_shared).shape[1]
    NA = np.asarray(W2).shape[2]
    Bl = B // N_CORES

    experts, inv = np.unique(route, return_inverse=True)
    groups = tuple(tuple(np.where(inv == s)[0].tolist())
                   for s in range(len(experts)))
    E = len(groups)
    G = len(groups[0])

    key = (B, A, D, H, NA, groups)
    nc = _cache.get(key)
    if nc is None:
        nc = _build(A, D, H, NA, Bl, groups)
        _cache[key] = nc

    x0t = np.ascontiguousarray(
        x0.reshape(N_CORES, Bl, A, D).transpose(0, 2, 3, 1)).astype(bf)
    wsb = np.asarray(W_shared, dtype=np.float32).astype(bf)
    bsf = np.asarray(b_shared, dtype=np.float32)
    w1g = np.ascontiguousarray(np.asarray(W1, np.float32)[experts]).astype(bf)
    b1g = np.ascontiguousarray(np.asarray(b1, np.float32)[experts])
    w2g = np.ascontiguousarray(np.asarray(W2, np.float32)[experts]).astype(bf)
    b2q = np.ascontiguousarray(
        np.tile(np.asarray(b2, np.float32)[experts], (1, G)))[:, :, None]

    in_maps = [
        dict(x0t=x0t[c], ws=wsb, bs=bsf,
             w1g=w1g, b1g=b1g, w2g=w2g, b2q=b2q)
        for c in range(N_CORES)
    ]
    import time
    last_err = None
    for attempt in range(3):
        try:
            res = run_bass_kernel_spmd(nc, in_maps,
                                       core_ids=list(range(N_CORES)),
                                       trace=_trace, tmpdir=_tmpdir)
            break
        except Exception as e:  # noqa: BLE001
            last_err = e
            time.sleep(5.0 * (attempt + 1))
    else:
        raise last_err
    kernel.last_exec_time_ns = res.exec_time_ns
    ytq = np.stack([res.results[c]["ytq"] for c in range(N_CORES)])
    agents_order = [a for g in groups for a in g]
    yp = ytq.reshape(N_CORES, E, G, NA, Bl).transpose(0, 4, 1, 2, 3)
    yp = yp.reshape(N_CORES, Bl, E * G, NA)
    y = np.empty((N_CORES, Bl, A, NA), np.float32)
    y[:, :, agents_order, :] = yp
    return np.ascontiguousarray(y).reshape(B, A, NA)


# revision 5
# speedup vs baseline: 1.0015x; 1.0015x over previous
"""Trainium2 Bass kernel for nn_DivTree (moe_routing) — v10.

v9 -> v10: the split final quad's two halves use different PSUM banks
(half 1 stalled 1.5us behind half 0's bias-add reading the shared bank).

v7 -> v9 (head-hole elimination):
- Group-0 bias triggers moved after the SECOND agent's x0 triggers on
  the scalar HWDGE queue (they were delaying x0(a1)'s second half by
  ~1.3us; b1/b2 aren't read until L2 evictions much later).
- x0 for the 2nd and 3rd agents split per k-piece across alternating
  HWDGE queues (like the first agent) instead of halves: with halves,
  L1's first matmul waited on a 2-piece DMA completion, leaving ~2us
  PE holes at 13-16us while the head is still bandwidth-bound.
"""

import numpy as np

P = 128
N_CORES = 8

_cache: dict = {}


def _build(A, D, H, NA, Bl, groups):
    import concourse.mybir as mybir
    import concourse.tile as tile
    from concourse import bacc
    from contextlib import ExitStack

    f32 = mybir.dt.float32
    bf16 = mybir.dt.bfloat16
    Relu = mybir.ActivationFunctionType.Relu
    E = len(groups)
    G = len(groups[0])
    assert all(len(g) == G for g in groups) and G * NA == P
    KD, KH, MH = D // P, H // P, H // P
    NB = Bl
    assert NB <= 512 and H % P == 0 and D % P == 0 and NA <= P

    nc = bacc.Bacc()
    x0t = nc.declare_dram_parameter("x0t", [A, D, Bl], bf16, isOutput=False)
    ws = nc.declare_dram_parameter("ws", [D, H], bf16, isOutput=False)
    bs = nc.declare_dram_parameter("bs", [H], f32, isOutput=False)
    w1g = nc.declare_dram_parameter("w1g", [E, H, H], bf16, isOutput=False)
    b1g = nc.declare_dram_parameter("b1g", [E, H], f32, isOutput=False)
    w2g = nc.declare_dram_parameter("w2g", [E, H, NA], bf16, isOutput=False)
    b2q = nc.declare_dram_parameter("b2q", [E, P, 1], f32, isOutput=False)
    ytq = nc.declare_dram_parameter("ytq", [E, P, Bl], f32, isOutput=True)

    with tile.TileContext(nc) as tc, ExitStack() as ctx:
        const = ctx.enter_context(tc.tile_pool(name="const", bufs=1))
        wpool = ctx.enter_context(tc.tile_pool(name="wexp", bufs=2))
        xpool = ctx.enter_context(tc.tile_pool(name="x0", bufs=4))
        x1pool = ctx.enter_context(tc.tile_pool(name="x1", bufs=3))
        hpool = ctx.enter_context(tc.tile_pool(name="h", bufs=6))
        opool = ctx.enter_context(tc.tile_pool(name="out", bufs=2))
        psum = ctx.enter_context(tc.tile_pool(name="ps", bufs=4, space="PSUM"))
        psum2 = ctx.enter_context(tc.tile_pool(name="ps2", bufs=3, space="PSUM"))
        psum3 = ctx.enter_context(tc.tile_pool(name="ps3", bufs=1, space="PSUM"))

        # PE warm-up through the DMA cold-start. Two alternating PSUM
        # banks keep the dummy matmuls back-to-back (100% duty) so the
        # HAM activity window reliably sees a busy PE and lifts the clock
        # to 2.4 GHz before real work; a single bank's WAW serialization
        # gave only ~50% duty and sometimes failed the busy threshold.
        dummy = const.tile([P, 128], bf16)
        nc.vector.memset(dummy[:], 0.0)
        dwa = psum.tile([64, 128], f32, tag="ps", name="warm_a")
        dwb = psum.tile([64, 128], f32, tag="ps", name="warm_b")
        for i in range(40):
            dps = dwa if i % 2 == 0 else dwb
            nc.tensor.matmul(dps[:], dummy[:, :64], dummy[:, :128],
                             start=True, stop=True)

        # head-critical loads: pair k = (x0 piece k, wsm tile k) split
        # across the two HWDGE queues so both members land together
        ws_r = ws.rearrange("(ks p) h -> p ks h", p=P)
        a0 = groups[0][0]
        x0_first = xpool.tile([P, KD, NB], bf16, tag="x0")
        x0_first_r = x0t[a0].rearrange("(ks p) b -> p ks b", p=P)
        wsm = [const.tile([P, KD, P], bf16, tag=f"wsm{ms}", name=f"wsm{ms}")
               for ms in range(MH)]
        for k in range(KD):
            e_x, e_w = ((nc.sync, nc.scalar) if k % 2 == 0
                        else (nc.scalar, nc.sync))
            e_x.dma_start(x0_first[:, k, :], x0_first_r[:, k, :])
            e_w.dma_start(wsm[k][:], ws_r[:, :, k * P:(k + 1) * P])
        bs_t = const.tile([P, MH], f32)
        nc.scalar.dma_start(bs_t[:], bs.rearrange("(ms p) -> p ms", p=P))

        def emit_l1(a, x0_t, diagonal=False):
            x1_t = x1pool.tile([P, MH, NB], bf16, tag="x1", name=f"x1_{a}")
            ps1 = [psum.tile([P, NB], f32, tag="ps", name=f"ps1_{a}_{ms}")
                   for ms in range(MH)]
            if diagonal:
                order = [(ks, ms) for k in range(max(KD, MH))
                         for ks in range(KD) for ms in range(MH)
                         if max(ks, ms) == k]
            else:
                order = [(ks, ms) for ms in range(MH) for ks in range(KD)]
            for ks, ms in order:
                nc.tensor.matmul(
                    ps1[ms][:], wsm[ms][:, ks, :], x0_t[:, ks, :],
                    start=(ks == 0), stop=(ks == KD - 1),
                )
            for ms in range(MH):
                if ms % 2:
                    nc.vector.tensor_scalar(
                        x1_t[:, ms, :], ps1[ms][:], bs_t[:, ms:ms + 1], 0.0,
                        mybir.AluOpType.add, mybir.AluOpType.max)
                else:
                    nc.scalar.activation(x1_t[:, ms, :], ps1[ms][:], Relu,
                                         bias=bs_t[:, ms:ms + 1])
            return x1_t

        def emit_l2(a, x1_t, wt):
            w1_t, b1_t = wt[0], wt[1]
            h_t = hpool.tile([P, MH, NB], bf16, tag="h", name=f"h_{a}")
            for ms in range(MH):
                ps2 = psum2.tile([P, NB], f32, tag="ps2", name=f"ps2_{a}_{ms}")
                for ks in range(KH):
                    nc.tensor.matmul(
                        ps2[:],
                        w1_t[:, ks, ms * P:(ms + 1) * P],
                        x1_t[:, ks, :],
                        start=(ks == 0), stop=(ks == KH - 1),
                    )
                if ms % 2:
                    nc.vector.tensor_scalar(
                        h_t[:, ms, :], ps2[:], b1_t[:, ms:ms + 1], 0.0,
                        mybir.AluOpType.add, mybir.AluOpType.max)
                else:
                    nc.scalar.activation(h_t[:, ms, :], ps2[:], Relu,
                                         bias=b1_t[:, ms:ms + 1])
            return h_t

        def emit_l3_quad(s, h_ts, wt, split=False):
            w2_t, b2_t = wt[2], wt[3]
            o_t = opool.tile([P, NB], f32, tag="o", name=f"o_{s}")
            # split=True pipelines the final quad in two batch halves so
            # the first half's bias-add + store overlap the second half's
            # matmuls (shrinks the post-matmul tail of the kernel). The
            # halves use DIFFERENT psum banks — with one bank, half 1's
            # matmuls stall ~1.5us behind half 0's bias-add reading it.
            nh = 2 if split else 1
            hw = NB // nh
            for hf in range(nh):
                cl, cr = hf * hw, (hf + 1) * hw
                pool = psum2 if hf else psum3
                tag = "ps2" if hf else "ps3"
                ps3 = pool.tile([P, hw], f32, tag=tag, name=f"ps3_{s}_{hf}")
                for ks in range(KH):
                    for j in range(G):
                        nc.tensor.matmul(
                            ps3[j * NA:(j + 1) * NA, :],
                            w2_t[:, ks, :],
                            h_ts[j][:, ks, cl:cr],
                            start=(ks == 0), stop=(ks == KH - 1),
                            tile_position=(0, j * NA),
                        )
                nc.vector.tensor_add(
                    o_t[:, cl:cr], ps3[:],
                    b2_t[:, 0:1].to_broadcast((P, hw)))
                nc.sync.dma_start(ytq[s][:, cl:cr], o_t[:, cl:cr])

        pending = None
        group_h = {}
        x0_tiles = [None] * A
        agents_flat = [a for g in groups for a in g]
        for s, agents in enumerate(groups):
            # SWDGE weight prefetch, paced: wait until an earlier agent's
            # x0 has landed so the deep W1 ring can't starve the HWDGE
            # queues during the head
            if s == 0:
                dep_t = x0_first
            else:
                dep_a = agents_flat[max(0, s * G - 1)]
                dep_t = x0_tiles[dep_a]
            w1_t = wpool.tile([P, KH, H], bf16, tag="w1", name=f"w1_{s}")
            w1_r = w1g[s].rearrange("(ks p) h -> p ks h", p=P)
            # WAW pacing: write one element of the destination tile from a
            # copy that reads the dep x0 tile's last-arriving slice. The
            # weight DMA (whole-tile write) must then order after the copy,
            # so the deep SWDGE ring can't start until the dep has landed.
            nc.gpsimd.tensor_copy(w1_t[0:1, 0, 0:1], dep_t[0:1, KD - 1, 0:1])
            nc.gpsimd.dma_start(w1_t[:], w1_r)
            w2_t = wpool.tile([P, KH, NA], bf16, tag="w2", name=f"w2_{s}")
            nc.gpsimd.tensor_copy(w2_t[0:1, 0, 0:1], dep_t[0:1, KD - 1, 0:1])
            nc.gpsimd.dma_start(
                w2_t[:], w2g[s].rearrange("(ks p) n -> p ks n", p=P))
            b1_t = wpool.tile([P, MH], f32, tag="b1", name=f"b1_{s}")
            b2_t = wpool.tile([P, 1], f32, tag="b2", name=f"b2_{s}")
            wt = (w1_t, b1_t, w2_t, b2_t)

            for ai, a in enumerate(agents):
                if a == a0:
                    x0_t = x0_first
                else:
                    x0_t = xpool.tile([P, KD, NB], bf16, tag="x0",
                                      name=f"x0_{a}")
                    x0_r = x0t[a].rearrange("(ks p) b -> p ks b", p=P)
                    if a in (agents_flat[1], agents_flat[2]):
                        # agents 2-3 land while the head is still DMA-bound:
                        # split per k-piece, alternating queues, so each
                        # piece unlocks L1 matmuls as soon as it arrives
                        # (a half-granular DMA made the first matmul wait
                        # for two pieces, leaving ~2us PE holes at ~13-16us)
                        for k in range(KD):
                            eng = nc.sync if k % 2 == 0 else nc.scalar
                            eng.dma_start(x0_t[:, k, :], x0_r[:, k, :])
                    else:
                        half = KD // 2
                        nc.sync.dma_start(x0_t[:, :half, :],
                                          x0_r[:, :half, :])
                        nc.scalar.dma_start(x0_t[:, half:, :],
                                            x0_r[:, half:, :])
                x0_tiles[a] = x0_t
                if ai == (1 if s == 0 else 0):
                    # bias loads ride behind the group's early x0 triggers
                    # (for group 0: after agent 1's, so they don't delay
                    # the head-critical x0(a1) second half on the scalar
                    # queue) — b1 is only read by L2 evictions much later
                    nc.scalar.dma_start(
                        b1_t[:], b1g[s].rearrange("(ms p) -> p ms", p=P))
                    nc.scalar.dma_start(b2_t[:], b2q[s])
                x1_t = emit_l1(a, x0_t, diagonal=(a == a0))
                if pending is not None:
                    pa, px1, pwt, pg, pj = pending
                    h_t = emit_l2(pa, px1, pwt)
                    group_h.setdefault(pg, []).append(h_t)
                    if pj == len(groups[pg]) - 1:
                        emit_l3_quad(pg, group_h.pop(pg), pwt)
                pending = (a, x1_t, wt, s, agents.index(a))
        pa, px1, pwt, pg, pj = pending
        h_t = emit_l2(pa, px1, pwt)
        group_h.setdefault(pg, []).append(h_t)
        emit_l3_quad(pg, group_h.pop(pg), pwt, split=True)

    nc.compile()
    return nc


def kernel(x0, W_shared, b_shared, W1, b1, W2, b2, route,
           _trace=False, _tmpdir=None):
    import ml_dtypes
    from concourse.bass_utils import run_bass_kernel_spmd

    bf = ml_dtypes.bfloat16
    x0 = np.asarray(x0, dtype=np.float32)
    route = np.asarray(route)

    B, A, D = x0.shape
    H = np.asarray(W_shared).shape[1]
    NA = np.asarray(W2).shape[2]
    Bl = B // N_CORES

    experts, inv = np.unique(route, return_inverse=True)
    groups = tuple(tuple(np.where(inv == s)[0].tolist())
                   for s in range(len(experts)))
    E = len(groups)
    G = len(groups[0])

    key = (B, A, D, H, NA, groups)
    nc = _cache.get(key)
    if nc is None:
        nc = _build(A, D, H, NA, Bl, groups)
        _cache[key] = nc

    x0t = np.ascontiguousarray(
        x0.reshape(N_CORES, Bl, A, D).transpose(0, 2, 3, 1)).astype(bf)
    wsb = np.asarray(W_shared, dtype=np.float32).astype(bf)
    bsf = np.asarray(b_shared, dtype=np.float32)
    w1g = np.ascontiguousarray(np.asarray(W1, np.float32)[experts]).astype(bf)
    b1g = np.ascontiguousarray(np.asarray(b1, np.float32)[experts])
    w2g = np.ascontiguousarray(np.asarray(W2, np.float32)[experts]).astype(bf)
    b2q = np.ascontiguousarray(
        np.tile(np.asarray(b2, np.float32)[experts], (1, G)))[:, :, None]

    in_maps = [
        dict(x0t=x0t[c], ws=wsb, bs=bsf,
             w1g=w1g, b1g=b1g, w2g=w2g, b2q=b2q)
        for c in range(N_CORES)
    ]
    import time
    last_err = None
    for attempt in range(3):
        try:
            res = run_bass_kernel_spmd(nc, in_maps,
                                       core_ids=list(range(N_CORES)),
                                       trace=_trace, tmpdir=_tmpdir)
            break
        except Exception as e:  # noqa: BLE001
            last_err = e
            time.sleep(5.0 * (attempt + 1))
    else:
        raise last_err
    kernel.last_exec_time_ns = res.exec_time_ns
    ytq = np.stack([res.results[c]["ytq"] for c in range(N_CORES)])
    agents_order = [a for g in groups for a in g]
    yp = ytq.reshape(N_CORES, E, G, NA, Bl).transpose(0, 4, 1, 2, 3)
    yp = yp.reshape(N_CORES, Bl, E * G, NA)
    y = np.empty((N_CORES, Bl, A, NA), np.float32)
    y[:, :, agents_order, :] = yp
    return np.ascontiguousarray(y).reshape(B, A, NA)
